# revision 2
# baseline (speedup 1.0000x reference)
"""Causal self-attention (B=4, T=2048, D=1024, H=16) on 8 trn2 NeuronCores.

Sharding: core c -> (batch b = c // 2, head-group g = c % 2). Each core runs
one batch element with 8 of the 16 heads: column-sharded Wq/Wk/Wv, row-sharded
Wp. Per-core output is a partial product of the output projection; the host
sums the two head-group partials per batch (bp is added on-device by group 0
via a broadcast input; group 1 gets zeros).

v2: software-pipelined per-chunk schedule. The baseline ran all projections
(PE-dense, ACT idle ~95us) then all attention (ACT-bound on exp, PE stalling).
Here chunk c's attention is interleaved -- at matmul granularity -- with chunk
c+1's projections and chunk c-1's output projection, so the PE has filler work
during every exp wait and the Act engine starts ~25us into the kernel.
All matmul operands are stored bf16 (PSUM accumulation stays fp32): same
1 cycle/row PE streaming as f32r but enables fast weight load, halves DMA,
and lets exp write bf16.

Layout (unchanged from baseline except per-chunk tiling):
  - x^T prepared host-side; projections stream xt chunks from HBM.
  - Q,K produced transposed and pair-of-heads stacked: qt/kt [128, PR, 512]
    per chunk (partitions = 2 heads x 64 dims) -> scores matmuls are K=64
    row-tiled pairs that run concurrently on the PE.
  - V stored per-head with an appended ones column vv [128, 4, H, 65] per
    chunk: U' = V'^T @ expS yields attention numerator (rows 0..63) and
    softmax denominator (row 64) in one accumulation group.
  - S^T = KT^T @ QT (no transposes anywhere); max-free softmax, exp with
    1/sqrt(64) folded in; causal masking via gpsimd affine_select post-exp.
  - U copied PSUM->SBUF immediately after the last AV matmul (releases the
    PSUM bank fast); the reciprocal of the denominator row is computed via
    the DMA-spread trick and applied as a bf16 2x-mode DVE multiply.
"""

import numpy as np
import ml_dtypes

import concourse.mybir as mybir
import concourse.tile as tile
from concourse import bacc
from concourse.bass_utils import run_bass_kernel_spmd

B, T, D, H_FULL = 4, 2048, 1024, 16
H = H_FULL // 2          # heads per core
HD = 64                  # head dim
DH = H * HD              # 512, per-core head width
P = 128
TT = T // P              # 16 t tiles
TC = T // 512            # 4 t chunks
KD = D // P              # 8 contraction tiles over D
PR = H // 2              # 4 head pairs
N_CORES = 8

F32 = mybir.dt.float32
BF16 = mybir.dt.bfloat16


class Filler:
    """Queue of deferred PE work (projections / output projection), emitted
    in small bites between attention matmul groups so the in-order PE queue
    always has independent work while ACT runs exp."""

    def __init__(self):
        self.units = []      # list of (key, generator) pairs
        self.cur = None
        self.cur_key = None
        self.mms = 0         # matmuls remaining (approximate pacing weight)
        self.acc = 0.0
        self.done_keys = set()

    def add(self, gen, n_mms, key=None):
        self.units.append((key, gen))
        self.mms += n_mms

    def pump(self, n):
        """Emit work until n matmuls have been issued (or queue empty)."""
        done = 0
        while done < n:
            if self.cur is None:
                if not self.units:
                    return
                self.cur_key, self.cur = self.units.pop(0)
            for kind, thunk in self.cur:
                thunk()
                if kind == "mm":
                    self.mms -= 1
                    done += 1
                    if done >= n:
                        break
            else:
                self.done_keys.add(self.cur_key)
                self.cur = None

    def ensure(self, key):
        """Emit whole units until the unit tagged `key` has been fully
        emitted. Emission order IS program order -- a consumer emitted
        before its producer reads stale data -- so anything an upcoming
        instruction reads must be forced out of the queue first."""
        if key in self.done_keys or not any(
                k == key for k, _ in self.units) and self.cur_key != key:
            return
        while key not in self.done_keys and (self.cur or self.units):
            if self.cur is None:
                self.cur_key, self.cur = self.units.pop(0)
            for kind, thunk in self.cur:
                thunk()
                if kind == "mm":
                    self.mms -= 1
            self.done_keys.add(self.cur_key)
            self.cur = None

    def pump_frac(self, quota):
        self.acc += quota
        n = int(self.acc)
        if n > 0:
            self.acc -= n
            self.pump(n)

    def drain(self):
        self.pump(1 << 30)


def build_nc():
    nc = bacc.Bacc(None, target_bir_lowering=False)

    xt = nc.dram_tensor("xt", [D, T], BF16, kind="ExternalInput")
    wq = nc.dram_tensor("wq", [D, DH], BF16, kind="ExternalInput")
    wk = nc.dram_tensor("wk", [D, DH], BF16, kind="ExternalInput")
    wv = nc.dram_tensor("wv", [D, DH], BF16, kind="ExternalInput")
    bq = nc.dram_tensor("bq", [P, PR], F32, kind="ExternalInput")
    bk = nc.dram_tensor("bk", [P, PR], F32, kind="ExternalInput")
    bvb = nc.dram_tensor("bvb", [P, DH], F32, kind="ExternalInput")
    wp = nc.dram_tensor("wp", [DH, D], BF16, kind="ExternalInput")
    bpb = nc.dram_tensor("bpb", [P, D], F32, kind="ExternalInput")
    y = nc.dram_tensor("y", [T, D], F32, kind="ExternalOutput")

    xt_r = xt.rearrange("(o p) t -> p o t", p=P)
    wq_r = wq.rearrange("(o p) f -> p o f", p=P)
    wk_r = wk.rearrange("(o p) f -> p o f", p=P)
    wv_r = wv.rearrange("(o p) f -> p o f", p=P)

    with tile.TileContext(nc) as tc:
        with (
            tc.tile_pool(name="persist", bufs=1) as pp,
            tc.tile_pool(name="xpool", bufs=2) as xpool,
            tc.tile_pool(name="epool", bufs=6) as epool,
            tc.tile_pool(name="upool", bufs=4) as upool,
            tc.tile_pool(name="rpool", bufs=2) as rpool,
            tc.tile_pool(name="ypool", bufs=3) as ypool,
            tc.tile_pool(name="work", bufs=2, space="PSUM") as work,
            tc.tile_pool(name="psS", bufs=2, space="PSUM") as psS,
            tc.tile_pool(name="psU0", bufs=1, space="PSUM") as psU0,
            tc.tile_pool(name="psU1", bufs=1, space="PSUM") as psU1,
        ):
            bq_s = pp.tile([P, PR], F32, name="bq_s")
            nc.sync.dma_start(bq_s[:], bq[:])
            bk_s = pp.tile([P, PR], F32, name="bk_s")
            nc.sync.dma_start(bk_s[:], bk[:])
            bvb_s = pp.tile([P, DH], F32, name="bvb_s")
            nc.sync.dma_start(bvb_s[:], bvb[:])
            bpb_s = pp.tile([P, D], F32, name="bpb_s")
            nc.sync.dma_start(bpb_s[:], bpb[:])

            wq_s = pp.tile([P, KD, DH], BF16, name="wq_s")
            wk_s = pp.tile([P, KD, DH], BF16, name="wk_s")
            wv_s = pp.tile([P, KD, DH], BF16, name="wv_s")
            wp_s = pp.tile([P, PR, D], BF16, name="wp_s")

            # per-chunk tensors (separate tiles -> exact dependency tracking
            # so interleaved chunks never falsely serialize); ot additionally
            # per head-pair so the output projection's pr-accumulation chain
            # can start as soon as the first pair is normalized
            qt = [pp.tile([P, PR, 512], BF16, name=f"qt{c}") for c in range(TC)]
            kt = [pp.tile([P, PR, 512], BF16, name=f"kt{c}") for c in range(TC)]
            vv = [pp.tile([P, 4, H, HD + 1], BF16, name=f"vv{c}")
                  for c in range(TC)]
            ot = [[pp.tile([P, 512], BF16, name=f"ot{c}_{pr}")
                   for pr in range(PR)] for c in range(TC)]
            for c in range(TC):
                nc.any.memset(vv[c][:, :, :, HD], 1.0)

            xt_tiles = {}

            # bulk transfers (x chunks, weights, y stores) are issued from
            # the gpsimd queue; the small latency-critical softmax DMAs
            # (r4/rb/om) stay on sync so they never sit behind a megabyte
            # prefetch in the same DMA queue
            def load_chunk_x(c):
                xt_tiles[c] = xpool.tile([P, KD, 512], BF16, name="xt_c",
                                         tag="xt")
                nc.gpsimd.dma_start(
                    xt_tiles[c][:], xt_r[:, :, c * 512:(c + 1) * 512])

            load_chunk_x(0)
            nc.gpsimd.dma_start(wq_s[:], wq_r[:])
            nc.gpsimd.dma_start(wk_s[:], wk_r[:])
            nc.gpsimd.dma_start(wv_s[:], wv_r[:])
            nc.gpsimd.dma_start(wp_s[:], wp.rearrange("(o p) f -> p o f", p=P))

            def proj_unit_gen(c, kind, m):
                """One projection subunit: 8 accumulating matmuls + bias add.
                kind: 0=Q, 1=K, 2=V(m = t4)."""
                xt_c = xt_tiles[c]
                pq = work.tile([P, 512], F32, name="pq", tag="pp")
                if kind < 2:
                    w_s = (wq_s, wk_s)[kind]
                    for dk in range(KD):
                        yield ("mm", (lambda dk=dk: nc.tensor.matmul(
                            pq[:],
                            w_s[:, dk, m * P:(m + 1) * P],
                            xt_c[:, dk, :],
                            start=(dk == 0),
                            stop=(dk == KD - 1),
                        )))
                    dst = (qt, kt)[kind]
                    b_s = (bq_s, bk_s)[kind]
                    yield ("free", (lambda: nc.vector.tensor_tensor(
                        out=dst[c][:, m, :],
                        in0=pq[:],
                        in1=b_s[:, m, None].to_broadcast((P, 512)),
                        op=mybir.AluOpType.add,
                    )))
                else:
                    for dk in range(KD):
                        yield ("mm", (lambda dk=dk: nc.tensor.matmul(
                            pq[:],
                            xt_c[:, dk, m * P:(m + 1) * P],
                            wv_s[:, dk, :],
                            start=(dk == 0),
                            stop=(dk == KD - 1),
                        )))
                    yield ("free", (lambda: nc.vector.tensor_tensor(
                        out=vv[c][:, m, :, 0:HD],
                        in0=pq.rearrange("p (h d) -> p h d", h=H),
                        in1=bvb_s.rearrange("p (h d) -> p h d", h=H),
                        op=mybir.AluOpType.add,
                    )))

            def outproj_unit_gen(c, tt4, n2):
                """One output-projection subunit: 4 accumulating matmuls +
                bias add + store. tt4 = t-tile within chunk, n2 = D half."""
                tt_ = 4 * c + tt4
                ts_ = slice(tt_ * P, (tt_ + 1) * P)
                ns = slice(n2 * 512, (n2 + 1) * 512)
                py = work.tile([P, 512], F32, name="py", tag="pp")
                for pr in range(PR):
                    yield ("mm", (lambda pr=pr: nc.tensor.matmul(
                        py[:],
                        ot[c][pr][:, tt4 * P:(tt4 + 1) * P],
                        wp_s[:, pr, ns],
                        start=(pr == 0),
                        stop=(pr == PR - 1),
                    )))
                yt = ypool.tile([P, 512], F32, name="yt", tag="yt")
                yield ("free", (lambda: nc.vector.tensor_tensor(
                    out=yt[:], in0=py[:], in1=bpb_s[:, ns],
                    op=mybir.AluOpType.add,
                )))
                yield ("free", (lambda: nc.gpsimd.dma_start(y[ts_, ns],
                                                            yt[:])))

            def add_proj_qk(fil, c):
                if c not in xt_tiles:
                    load_chunk_x(c)
                for kind in range(2):
                    for m in range(4):
                        fil.add(proj_unit_gen(c, kind, m), KD)

            def add_proj_v(fil, c):
                for m in range(4):
                    fil.add(proj_unit_gen(c, 2, m), KD, key=("v", c, m))

            def add_outproj(fil, c):
                for tt4 in range(4):
                    for n2 in range(2):
                        fil.add(outproj_unit_gen(c, tt4, n2), PR)

            def attn_chunk(c, fil):
                ntk = 4 * c + 4
                npairs = (ntk // 2) * PR
                quota = fil.mms / max(npairs, 1)
                for hp in range(PR):
                    ups = [
                        (psU0 if j == 0 else psU1).tile(
                            [HD + 1, 512], F32, name=f"up{j}", tag=f"u{j}")
                        for j in (0, 1)
                    ]
                    for tp in range(0, ntk, 2):
                        diag = tp >= 4 * c
                        r0 = P * (tp - 4 * c) if diag else 0
                        sps, ets = [], []
                        for i in (0, 1):
                            sps.append(psS.tile(
                                [P, 2, 512], F32, name="sp", tag="s"))
                            ets.append(epool.tile(
                                [P, 2, 512], BF16, name="et", tag="e"))
                        for i in (0, 1):
                            t = tp + i
                            tc_, t4 = t // 4, t % 4
                            for j in (0, 1):
                                # j=0 rows 0-63, j=1 rows 64-127: disjoint
                                # row groups run concurrently on the PE
                                pb = 64 * j
                                nc.tensor.matmul(
                                    sps[i][:, j, r0:512],
                                    kt[tc_][pb:pb + 64, hp,
                                            t4 * P:(t4 + 1) * P],
                                    qt[c][pb:pb + 64, hp, r0:512],
                                    start=True,
                                    stop=True,
                                )
                        for i in (0, 1):
                            nc.scalar.activation(
                                ets[i][:, :, r0:512], sps[i][:, :, r0:512],
                                mybir.ActivationFunctionType.Exp,
                                scale=float(1.0 / np.sqrt(HD)),
                            )
                            if diag:
                                # same mask for both heads (coeff 0 on j)
                                nc.gpsimd.affine_select(
                                    out=ets[i][:, :, r0:512],
                                    in_=ets[i][:, :, r0:512],
                                    compare_op=mybir.AluOpType.is_ge,
                                    fill=0.0,
                                    base=-P * i,
                                    pattern=[[0, 2], [1, 512 - r0]],
                                    channel_multiplier=-1,
                                )
                        # filler between scores/exp and the dependent AV
                        # matmuls: the PE would otherwise stall here
                        fil.pump_frac(quota)
                        for i in (0, 1):
                            # producers of vv must be emitted before the AV
                            # matmuls that read them (emission order is
                            # program order)
                            t = tp + i
                            fil.ensure(("v", t // 4, t % 4))
                        for i in (0, 1):
                            t = tp + i
                            tc_, t4 = t // 4, t % 4
                            for j in (0, 1):
                                nc.tensor.matmul(
                                    ups[j][:, r0:512],
                                    vv[tc_][:, t4, 2 * hp + j, :],
                                    ets[i][:, j, r0:512],
                                    start=(t == 0),
                                    stop=(t == ntk - 1),
                                )
                    # release PSUM fast: copy U' (numerator + denom row) to
                    # SBUF, then normalize from SBUF off the critical path
                    lp = nc.allow_low_precision(
                        reason="bf16 softmax normalization; rel tol 2e-2")
                    lp.__enter__()
                    for j in (0, 1):
                        uu = upool.tile([HD + 1, 512], BF16, name="uu",
                                        tag=f"uu{j}")
                        nc.vector.tensor_copy(uu[:], ups[j][:])
                        # reciprocal of denom row: spread [1,512]->[128,4]
                        # so the 8-cyc/elem DVE reciprocal uses 128 lanes
                        r4 = rpool.tile([P, 4], BF16, name="r4", tag="r4")
                        nc.sync.dma_start(r4[:], uu[HD:HD + 1, :])
                        r4r = rpool.tile([P, 4], BF16, name="r4r", tag="r4r")
                        nc.vector.reciprocal(r4r[:], r4[:])
                        rb = rpool.tile([1, 512], BF16, name="rb", tag="rb")
                        nc.sync.dma_start(rb[:], r4r[:])
                        bc = rpool.tile([64, 512], BF16, name="bc",
                                        tag=f"bc{j}")
                        nc.gpsimd.partition_broadcast(bc[:], rb[0:1, :])
                        if j == 0:
                            nc.vector.tensor_tensor(
                                out=ot[c][hp][0:64, :], in0=uu[0:64, :],
                                in1=bc[:], op=mybir.AluOpType.mult,
                            )
                        else:
                            om = rpool.tile([64, 512], BF16, name="om",
                                            tag="om")
                            nc.vector.tensor_tensor(
                                out=om[:], in0=uu[0:64, :], in1=bc[:],
                                op=mybir.AluOpType.mult,
                            )
                            nc.sync.dma_start(ot[c][hp][64:128, :], om[:])
                    lp.__exit__(None, None, None)

            # ---------------- schedule ----------------
            # chunk-0 Q/K projections run up front (attention needs them);
            # chunk-0 V projections lead the filler queue -- the scheduler is
            # dependency-driven, so AV matmuls simply wait for their V tile
            # if pacing ever under-delivers.
            fil = Filler()
            add_proj_qk(fil, 0)
            fil.drain()
            add_proj_v(fil, 0)
            for c in range(TC):
                if c + 1 < TC:
                    add_proj_qk(fil, c + 1)
                    add_proj_v(fil, c + 1)
                if c >= 1:
                    add_outproj(fil, c - 1)
                attn_chunk(c, fil)
                fil.drain()
            add_outproj(fil, TC - 1)
            fil.drain()

    nc.compile()
    return nc


_NC_CACHE = None


def _get_nc():
    global _NC_CACHE
    if _NC_CACHE is None:
        _NC_CACHE = build_nc()
    return _NC_CACHE


def _shard_inputs(x, Wq, bq, Wk, bk, Wv, bv, Wp, bp):
    """Build the 8 per-core input maps."""
    bf16 = ml_dtypes.bfloat16
    x = np.asarray(x, dtype=np.float32)
    ca = np.ascontiguousarray
    in_maps = []
    for core in range(N_CORES):
        b, g = core // 2, core % 2
        cols = slice(g * DH, (g + 1) * DH)
        bq_g = np.asarray(bq[cols], np.float32).reshape(PR, P).T
        bk_g = np.asarray(bk[cols], np.float32).reshape(PR, P).T
        bv_g = np.broadcast_to(np.asarray(bv[cols], np.float32), (P, DH))
        if g == 0:
            bp_b = np.broadcast_to(np.asarray(bp, np.float32), (P, D))
        else:
            bp_b = np.zeros((P, D), np.float32)
        in_maps.append({
            "xt": ca(x[b].T.astype(bf16)),
            "wq": ca(np.asarray(Wq, np.float32)[:, cols].astype(bf16)),
            "wk": ca(np.asarray(Wk, np.float32)[:, cols].astype(bf16)),
            "wv": ca(np.asarray(Wv, np.float32)[:, cols].astype(bf16)),
            "bq": ca(bq_g),
            "bk": ca(bk_g),
            "bvb": ca(bv_g),
            "wp": ca(np.asarray(Wp, np.float32)[cols, :].astype(bf16)),
            "bpb": ca(bp_b),
        })
    return in_maps


def run_sharded(inputs, trace=False):
    """Run on 8 cores; returns (full_output, BassKernelResults)."""
    nc = _get_nc()
    in_maps = _shard_inputs(**inputs)
    res = run_bass_kernel_spmd(
        nc, in_maps, core_ids=list(range(N_CORES)), trace=trace
    )
    out = np.empty((B, T, D), np.float32)
    for b in range(B):
        out[b] = res.results[2 * b]["y"] + res.results[2 * b + 1]["y"]
    return out, res


def kernel(**inputs) -> np.ndarray:
    out, _ = run_sharded(inputs)
    return out
